# revision 34
# baseline (speedup 1.0000x reference)
"""GCN (4-layer, DGL GraphConv norm='both' + BatchNorm + residual + mean
readout + MLP) on 8 Trainium2 NeuronCores via Bass/Tile.

Strategy: nodes are dst-sharded 8 ways (12544 padded rows/core). Each core
aggregates its own nodes' in-edges: feature rows are pulled with 4 parallel
`dma_gather` streams (one per SWDGE queue, sources split into 4 int16-indexable
chunks of the replicated bf16 node table), and the segment-sum over sorted
edges is a PE matmul against a pure 0/1 one-hot built on DVE (iota `is_equal`
dstcol, bf16). The DGL norm weight is split: 1/sqrt(deg_out) is folded into
the replicated gather table at transpose time (per-partition scale on the
Activation engine), 1/sqrt(deg_in) multiplies the aggregated block once per
dense group. All heavy matmuls (edge aggregation, W, transposes) run in bf16
(1 PE cycle/row vs 4 for fp32); BatchNorm statistics and the tiny MLP stay
fp32. x (residual) and hnT (pre-BN) live in SBUF for the whole kernel; only
the node table round-trips through HBM for the AllGather. Readout is fused
into the last layer's transpose pass (graph-membership one-hot matmul), then
an AllReduce and the 3-layer MLP. Degrees/norm weights, edge sorting/padding
and all shapes are host-side preprocessing; per-(chunk,window) edge-chunk
counts are padded to the max over cores so one program serves all 8 cores.
"""

import os
import sys
import types
import numpy as np

# ---------------------------------------------------------------- problem dims
N = 100000
E_FULL = 1600000
G = 128
HID = 128
L = 4
NC = 8
EPS = 1e-5
V = N // NC                 # 12500 real nodes per core
WPC = (V + 127) // 128      # 98 windows per core
VP = WPC * 128              # 12544 padded nodes per core
NPAD = NC * VP              # 100352 padded global rows
NCHUNK = 4
# table chunks = node QUARTERS (all 8 cores' quarter-q rows) so the
# AllGather can be split per quarter and pipelined with the next layer's
# gathers; quarter window counts chosen so each chunk stays int16-indexable
QW = [25, 25, 24, 24]               # windows per quarter
QSW = [0, 25, 50, 74]               # first window of quarter
QR = [3200, 3200, 3072, 3072]       # rows per core per quarter
QSR = [0, 3200, 6400, 9472]         # first row of quarter (per core)
CHQ = [25600, 25600, 24576, 24576]  # chunk rows (8 cores x QR, < 2**15)
GRP = 4                     # windows per dense group
OHB = 8                     # chunks per one-hot batch
GN = 1024                   # indices per dma_gather call


# ------------------------------------------------------------- compile patches
def _apply_patches():
    """This walrus build accepts only one sync-wait per instruction; hoist
    extra waits emitted by the Tile scheduler onto single-wait NoOps."""
    import concourse.mybir as mb
    from concourse.tile import TileContext, ScopedClock

    if getattr(TileContext, "_gcn_patched", False):
        return
    orig = TileContext._commit_and_lower

    def _split_waits(self, inst, *args):
        si = getattr(inst, "sync_info", None)
        if si is not None:
            waits = list(si.on_wait or [])
            if len(waits) > 1:
                for w in waits[:-1]:
                    nop = mb.InstNoOp(
                        name=self.nc.get_next_instruction_name(), ins=[], outs=[]
                    )
                    nop.engine = inst.engine
                    nop.sync_info = mb.SyncInfo(on_wait=[w], on_update=[])
                    orig(self, nop, *args)
                inst.sync_info = mb.SyncInfo(
                    on_wait=[waits[-1]], on_update=list(si.on_update or [])
                )
        return orig(self, inst, *args)

    def _drain_and_barrier(self, tick_clock, wait_clock):
        nop = self.nc.sync.nop(nofuse=True)
        inst = nop.ins
        wait_clock.add_sem_waits(inst, ScopedClock({None: tick_clock.global_clock}))
        si = inst.sync_info
        waits = list(si.on_wait) if si is not None else []
        inst.sync_info = mb.SyncInfo(on_wait=waits[:1], on_update=[])
        for w in waits[1:]:
            n2 = self.nc.sync.nop(nofuse=True)
            n2.ins.sync_info = mb.SyncInfo(on_wait=[w], on_update=[])
        self.nc.sync.drain()
        self.nc.all_engine_barrier()
        assert self.sems is not None
        popped = self.nc._tile_sem_poison_stack.pop()
        assert popped is self._sem_poison
        self.nc.clear_and_free_semaphores(list(self.sems.allocated().values()))
        self.nc.all_engine_barrier()

    TileContext._commit_and_lower = _split_waits
    TileContext._drain_and_barrier = _drain_and_barrier
    TileContext._gcn_patched = True


# --------------------------------------------------------- host preprocessing
def build_plan(src, dst, graph_id):
    src = np.asarray(src).astype(np.int64)
    dst = np.asarray(dst).astype(np.int64)
    graph_id = np.asarray(graph_id).astype(np.int64)

    deg_out = np.bincount(src, minlength=N).astype(np.float64)
    deg_in = np.bincount(dst, minlength=N).astype(np.float64)
    inv_do = (1.0 / np.sqrt(np.maximum(deg_out, 1.0))).astype(np.float32)
    inv_di = (1.0 / np.sqrt(np.maximum(deg_in, 1.0))).astype(np.float32)

    rsrc = src // V
    isrc = src % V
    wsrc = isrc // 128
    chunk = np.digitize(wsrc, QSW[1:] + [WPC])       # source quarter 0..3
    idxloc = (rsrc * np.array(QR)[chunk]
              + (isrc - np.array(QSR)[chunk])).astype(np.int16)
    srcrow = chunk * (1 << 20) + idxloc.astype(np.int64)  # sort key only
    core = dst // V
    dloc = dst - core * V
    win = dloc // 128
    dcol = (dloc % 128).astype(np.float32)

    # bucket edges per (core, chunk, window); pack slots tightly: each
    # (chunk, window) bucket gets max-over-cores slots (>=128 so a 128-slot
    # matmul chunk straddles at most one window boundary).
    counts = np.zeros((NC, NCHUNK, WPC), np.int64)
    np.add.at(counts, (core, chunk, win), 1)
    P = np.maximum(counts.max(axis=0), 128)          # [NCHUNK, WPC] slots
    S = np.zeros_like(P)
    S[:, 1:] = np.cumsum(P, axis=1)[:, :-1]          # slot start per window
    SL = P.sum(axis=1)                               # used slots per stream
    Cc = (SL + 127) // 128                           # matmul chunks per stream
    CE = Cc * 128                                    # chunk-covered slots
    EcP = ((CE + GN - 1) // GN) * GN                 # gather-padded length
    Ctot = int(Cc.sum())
    kf = S // 128                                    # first chunk per (c,w)
    kl = (S + P - 1) // 128                          # last chunk per (c,w)
    maxspan = int((kl - kf + 1).max())

    # per-core packed arrays
    order = np.lexsort((srcrow, win, chunk, core))  # stable grouping
    s_src = idxloc[order]
    s_dcol = dcol[order]
    # boundaries per (core, chunk, window)
    starts = np.zeros((NC, NCHUNK, WPC), np.int64)
    flatc = counts.reshape(-1)
    starts.reshape(-1)[1:] = np.cumsum(flatc)[:-1]

    # gather calls are emitted stream-major; assign queues round-robin over
    # the global call sequence so consecutive calls land on different SWDGE
    # descriptor rings (a ring only fits one call's descriptors)
    NCALL = EcP // GN
    cumcall = np.concatenate([[0], np.cumsum(NCALL)[:-1]])
    totcall = int(NCALL.sum())
    GC16 = GN // 16
    T16 = ((totcall + 3) // 4) * GC16
    idx16 = np.full((NC, 128, T16), -1, np.int16)
    dcol_p = np.empty((NC, 128, Ctot), np.float32)

    for r in range(NC):
        st_d = []
        for c in range(NCHUNK):
            # pad slots keep idx 0 (real row, weight 0) — a skipped gather
            # leaves stale SBUF bytes which can be NaN and 0*NaN = NaN in PE
            ii = np.zeros(int(EcP[c]), np.int16)
            delta = np.full(int(CE[c]), -1.0, np.float32)
            for w in range(WPC):
                cnt = counts[r, c, w]
                s0 = starts[r, c, w]
                pos = int(S[c, w])
                ii[pos:pos + cnt] = s_src[s0:s0 + cnt]
                # window parity offset keeps straddling chunks unambiguous
                delta[pos:pos + cnt] = s_dcol[s0:s0 + cnt] + 128.0 * (w % 2)
            ii[int(CE[c]):] = -1                       # gather tail skip
            st_d.append(delta)
            for j in range(int(NCALL[c])):
                gcall = int(cumcall[c]) + j
                q, bp = gcall % 4, gcall // 4
                a = ii[j * GN:(j + 1) * GN].reshape(-1, 16).T   # [16, GC16]
                idx16[r, 32 * q:32 * q + 32, bp * GC16:(bp + 1) * GC16] = \
                    np.tile(a, (2, 1))
        dall = np.concatenate(st_d)
        dcol_p[r] = dall.reshape(-1, 128).T

    # graph ids per core window layout [128, WPC], pad = -1
    gid_p = np.full((NC, 128, WPC), -1.0, np.float32)
    # per-node norm factors, per core
    invdo_p = np.ones((NC, 128, WPC), np.float32)    # node-major [p, w]
    invdi_p = np.ones((NC, VP), np.float32)          # per local node col
    for r in range(NC):
        g = graph_id[r * V:(r + 1) * V].astype(np.float32)
        gp = np.full(VP, -1.0, np.float32)
        gp[:V] = g
        gid_p[r] = gp.reshape(WPC, 128).T
        do = np.ones(VP, np.float32)
        do[:V] = inv_do[r * V:(r + 1) * V]
        invdo_p[r] = do.reshape(WPC, 128).T
        di = np.ones(VP, np.float32)
        di[:V] = inv_di[r * V:(r + 1) * V]
        invdi_p[r] = di

    cnts = np.maximum(np.bincount(graph_id, minlength=G).astype(np.float32), 1.0)
    recip = np.tile((1.0 / cnts)[None, :], (128, 1)).astype(np.float32)

    return dict(
        Cc=Cc, CE=CE, EcP=EcP, Ctot=Ctot, kf=kf, kl=kl, maxspan=maxspan,
        NCALL=NCALL, cumcall=cumcall, T16=T16,
        idx16=idx16, dcol=dcol_p, gid=gid_p, recip=recip,
        invdo=invdo_p, invdi=invdi_p,
    )


# ------------------------------------------------------------ program builder
def build_program(plan):
    _apply_patches()
    import concourse.bacc as bacc
    import concourse.mybir as mybir
    from concourse.tile import TileContext

    f32 = mybir.dt.float32
    bf16 = mybir.dt.bfloat16
    i16 = mybir.dt.int16
    AX = mybir.AxisListType.X
    OP = mybir.AluOpType
    AF = mybir.ActivationFunctionType

    Cc = plan["Cc"]
    CE = plan["CE"]
    EcP = plan["EcP"]
    Ctot = int(plan["Ctot"])
    kf = plan["kf"]
    kl = plan["kl"]
    maxspan = int(plan["maxspan"])
    NCALL = plan["NCALL"]
    cumcall = plan["cumcall"]
    T16 = int(plan["T16"])
    cbase = np.concatenate([[0], np.cumsum(Cc)[:-1]])      # chunk col base
    NGRP = (WPC + GRP - 1) // GRP
    GNI = GN // 128          # gathered sub-chunks per gather call
    GC16 = GN // 16          # idx16 cols per gather call

    nc = bacc.Bacc("TRN2", target_bir_lowering=False, debug=False,
                   enable_asserts=False, num_devices=NC, num_swdge_queues=4)

    # ---- external inputs
    ext = {}

    def inp(name, shape, dt=f32):
        ext[name] = nc.dram_tensor(name, list(shape), dt, kind="ExternalInput")
        return ext[name]

    hT_sh = inp("hT_shard", [128, VP], bf16)
    idx_d = inp("idx16", [128, T16], i16)
    dcol_d = inp("dcol", [128, Ctot], bf16)
    invdi_d = inp("invdi", [128, VP], bf16)
    invdo_d = inp("invdo", [128, WPC])
    gid_d = inp("gid", [128, WPC])
    recip_d = inp("recip", [128, G])
    iota_d = inp("iota", [128, 128], bf16)
    iotaA_d = inp("iotaA", [128, maxspan * 128], bf16)
    iotaB_d = inp("iotaB", [128, maxspan * 128], bf16)
    ident_d = inp("ident", [128, 128], bf16)
    wemb_d = inp("W_embed", [HID, HID], bf16)
    bemb_d = inp("b_embed", [HID, 1])
    wl_d = [inp(f"Wl{i}", [HID, HID], bf16) for i in range(L)]
    gam_d = inp("gammas", [HID, L])
    bet_d = inp("betas", [HID, L])
    w1_d = inp("W1", [128, 64])
    b1_d = inp("b1", [64, 1])
    w2_d = inp("W2", [64, 32])
    b2_d = inp("b2", [32, 1])
    w3_d = inp("W3", [32, 10])
    b3_d = inp("b3", [10, 1])

    out_d = nc.dram_tensor("out", [10, G], f32, kind="ExternalOutput")

    # ---- internal DRAM (split per quarter so each AllGather piece can fire
    # as soon as its windows are written, and gathers start per piece)
    x_all = [[nc.dram_tensor(f"x_all{i}_{q}", [CHQ[q], HID], bf16,
                             addr_space="Shared") for q in range(4)]
             for i in range(L)]
    xr_b = [[nc.dram_tensor(f"xr{i}_{q}", [QR[q], HID], bf16)
             for q in range(4)] for i in range(L)]
    ar_in = [nc.dram_tensor(f"arin{i}", [128, 2], f32) for i in range(L)]
    ar_out = [nc.dram_tensor(f"arout{i}", [128, 2], f32, addr_space="Shared")
              for i in range(L)]
    hg_in = nc.dram_tensor("hgin", [128, G], f32)
    hg_out = nc.dram_tensor("hgout", [128, G], f32, addr_space="Shared")

    RG = [list(range(NC))]

    with TileContext(nc) as tc:
        cp = tc.alloc_tile_pool(name="const", bufs=1)
        wp = tc.alloc_tile_pool(name="work", bufs=3)
        mp = tc.alloc_tile_pool(name="moh", bufs=4)
        gpool = tc.alloc_tile_pool(name="gsx", bufs=8)
        pp = tc.alloc_tile_pool(name="ps", bufs=3, space="PSUM")
        pp2 = tc.alloc_tile_pool(name="ps2", bufs=2, space="PSUM")
        pp3 = tc.alloc_tile_pool(name="ps3", bufs=1, space="PSUM")
        pt = tc.alloc_tile_pool(name="pst", bufs=2, space="PSUM")

        def load_const(name, dram, shape, dt=f32):
            t = cp.tile(list(shape), dt, tag=name)
            nc.sync.dma_start(out=t[:], in_=dram[:, :])
            return t

        idx_t = load_const("idx", idx_d, [128, T16], i16)
        dcol_t = load_const("dcol", dcol_d, [128, Ctot], bf16)
        invdi_t = load_const("invdi", invdi_d, [128, VP], bf16)
        invdo_t = load_const("invdo", invdo_d, [128, WPC])
        gid_t = load_const("gid", gid_d, [128, WPC])
        recip_t = load_const("recip", recip_d, [128, G])
        iota_t = load_const("iota", iota_d, [128, 128], bf16)
        iotaA_t = load_const("iotaA", iotaA_d, [128, maxspan * 128], bf16)
        iotaB_t = load_const("iotaB", iotaB_d, [128, maxspan * 128], bf16)
        ident_t = load_const("ident", ident_d, [128, 128], bf16)
        wemb_t = load_const("wemb", wemb_d, [HID, HID], bf16)
        bemb_t = load_const("bemb", bemb_d, [HID, 1])
        wl_t = [load_const(f"wl{i}", wl_d[i], [HID, HID], bf16) for i in range(L)]
        gam_t = load_const("gam", gam_d, [HID, L])
        bet_t = load_const("bet", bet_d, [HID, L])
        w1_t = load_const("w1", w1_d, [128, 64])
        b1_t = load_const("b1", b1_d, [64, 1])
        w2_t = load_const("w2", w2_d, [64, 32])
        b2_t = load_const("b2", b2_d, [32, 1])
        w3_t = load_const("w3", w3_d, [32, 10])
        b3_t = load_const("b3", b3_d, [10, 1])

        eps_t = cp.tile([128, 1], f32, tag="eps")
        nc.vector.memset(eps_t[:], EPS)
        hnT = cp.tile([128, VP], bf16, tag="hnT")
        xT = cp.tile([128, VP], bf16, tag="xT")
        aggT = cp.tile([128, VP], bf16, tag="aggT")
        ssum = cp.tile([128, NGRP], f32, tag="ssum")
        ssq = cp.tile([128, NGRP], f32, tag="ssq")

        def qof(w):
            return 0 if w < QSW[1] else (1 if w < QSW[2] else
                                         (2 if w < QSW[3] else 3))

        def xr_write(k, w, nmt):
            q = qof(w)
            ro = (w - QSW[q]) * 128
            nc.sync.dma_start(out=xr_b[k][q][ro:ro + 128, :], in_=nmt[:])

        # ---------------------------------------------------------- embedding
        for gi in range(NGRP):
            gw = min(GRP, WPC - gi * GRP)
            wid = gw * 128
            sl = slice(gi * GRP * 128, gi * GRP * 128 + wid)
            hTg = wp.tile([128, GRP * 128], bf16, tag="hTg")
            nc.sync.dma_start(out=hTg[:, :wid], in_=hT_sh[:, sl])
            x0p = pp2.tile([128, GRP * 128], f32, tag="hnp")
            nc.tensor.matmul(out=x0p[:, :wid], lhsT=wemb_t[:], rhs=hTg[:, :wid],
                             start=True, stop=True)
            nc.vector.tensor_scalar_add(out=xT[:, sl], in0=x0p[:, :wid],
                                        scalar1=bemb_t[:, 0:1])
            for wi in range(gw):
                w = gi * GRP + wi
                tpp = pt.tile([128, 128], bf16, tag="tp16")
                nc.tensor.transpose(out=tpp[:],
                                    in_=xT[:, w * 128:(w + 1) * 128],
                                    identity=ident_t[:])
                nmt = wp.tile([128, 128], bf16, tag="nmb")
                nc.scalar.activation(out=nmt[:], in_=tpp[:], func=AF.Copy,
                                     scale=invdo_t[:, w:w + 1])
                xr_write(0, w, nmt)

        # --------------------------------------------------------- GCN layers
        for l in range(L):
            call_list = []
            for c in range(4):
                for j in range(int(NCALL[c])):
                    g = int(cumcall[c]) + j
                    call_list.append((c, j, g % 4, g // 4))
            gpos = [0]
            gtiles = {}

            def ensure_gather(gid):
                while gpos[0] <= gid:
                    c, j, qn, bp = call_list[gpos[0]]
                    t = gpool.tile([128, GNI, 128], bf16, tag="gt")
                    cnt = min(GN, int(CE[c]) - j * GN)
                    nc.gpsimd.dma_gather(
                        out_ap=t[:],
                        in_ap=x_all[l][c][:, :],
                        idxs_ap=idx_t[:, bp * GC16:(bp + 1) * GC16],
                        num_idxs=GN, num_idxs_reg=cnt, elem_size=HID,
                        queue_num=qn)
                    gtiles[gpos[0]] = t
                    gtiles.pop(gpos[0] - 8, None)
                    gpos[0] += 1

            hgp = None
            if l == L - 1:
                hgp = pp3.tile([128, G], f32, tag="hgp")

            # stream-major: each stream's AllGather piece is issued right
            # before its first gather so Pool-queue order never blocks a
            # ready gather behind a not-yet-ready collective
            for c in range(4):
                nc.gpsimd.collective_compute(
                    "AllGather", mybir.AluOpType.bypass, replica_groups=RG,
                    ins=[xr_b[l][c].ap().opt()], outs=[x_all[l][c].ap().opt()])
                for w in range(WPC):
                    a, b = int(kf[c][w]), int(kl[c][w])
                    nb = b - a + 1
                    ensure_gather(int(cumcall[c]) + b // GNI)
                    m = mp.tile([128, maxspan * 128], bf16, tag="moh")
                    m3 = m[:].rearrange("p (b c) -> p b c", c=128)[:, 0:nb, :]
                    io = iotaA_t if w % 2 == 0 else iotaB_t
                    i3 = io[:].rearrange("p (b c) -> p b c", c=128)[:, 0:nb, :]
                    col0 = int(cbase[c]) + a
                    dsl = dcol_t[:, col0:col0 + nb].to_broadcast([128, nb, 128])
                    nc.vector.tensor_tensor(out=m3, in0=i3, in1=dsl,
                                            op=OP.is_equal)
                    psw = pp.tile([128, 128], f32, tag="mm128")
                    for k in range(a, b + 1):
                        gid = int(cumcall[c]) + k // GNI
                        nc.tensor.matmul(
                            out=psw[:], lhsT=gtiles[gid][:, k % GNI, :],
                            rhs=m[:, (k - a) * 128:(k - a + 1) * 128],
                            start=(k == a), stop=(k == b))
                    wcols = slice(w * 128, (w + 1) * 128)
                    if c == 0:
                        nc.scalar.activation(out=aggT[:, wcols], in_=psw[:],
                                             func=AF.Copy)
                    else:
                        nc.vector.tensor_tensor(out=aggT[:, wcols], in0=psw[:],
                                                in1=aggT[:, wcols], op=OP.add)
                    if c == 3:
                        gi, wi = w // GRP, w % GRP
                        gw = min(GRP, WPC - gi * GRP)
                        if wi == gw - 1:
                            wid = gw * 128
                            sl = slice(gi * GRP * 128, gi * GRP * 128 + wid)
                            agg4 = wp.tile([128, GRP * 128], bf16, tag="agg4")
                            nc.vector.tensor_tensor(out=agg4[:, :wid],
                                                    in0=aggT[:, sl],
                                                    in1=invdi_t[:, sl],
                                                    op=OP.mult)
                            hnp = pp2.tile([128, GRP * 128], f32, tag="hnp")
                            nc.tensor.matmul(out=hnp[:, :wid], lhsT=wl_t[l][:],
                                             rhs=agg4[:, :wid],
                                             start=True, stop=True)
                            nc.scalar.activation(out=hnT[:, sl],
                                                 in_=hnp[:, :wid],
                                                 func=AF.Copy,
                                                 accum_out=ssum[:, gi:gi + 1])
                            sq = wp.tile([128, GRP * 128], bf16, tag="sq")
                            nc.scalar.activation(out=sq[:, :wid],
                                                 in_=hnT[:, sl],
                                                 func=AF.Square,
                                                 accum_out=ssq[:, gi:gi + 1])

            # ----- BN stats + AllReduce
            stat_t = wp.tile([128, 2], f32, tag="stat")
            nc.vector.reduce_sum(out=stat_t[:, 0:1], in_=ssum[:], axis=AX)
            nc.vector.reduce_sum(out=stat_t[:, 1:2], in_=ssq[:], axis=AX)
            nc.sync.dma_start(out=ar_in[l][:, :], in_=stat_t[:])
            nc.gpsimd.collective_compute(
                "AllReduce", mybir.AluOpType.add, replica_groups=RG,
                ins=[ar_in[l].ap().opt()], outs=[ar_out[l].ap().opt()])
            st2 = wp.tile([128, 2], f32, tag="st2")
            nc.sync.dma_start(out=st2[:], in_=ar_out[l][:, :])
            mu = wp.tile([128, 1], f32, tag="mu")
            nc.vector.tensor_scalar_mul(out=mu[:], in0=st2[:, 0:1],
                                        scalar1=1.0 / N)
            var = wp.tile([128, 1], f32, tag="var")
            nc.vector.tensor_scalar_mul(out=var[:], in0=st2[:, 1:2],
                                        scalar1=1.0 / N)
            musq = wp.tile([128, 1], f32, tag="musq")
            nc.vector.tensor_tensor(out=musq[:], in0=mu[:], in1=mu[:],
                                    op=OP.mult)
            nc.vector.tensor_tensor(out=var[:], in0=var[:], in1=musq[:],
                                    op=OP.subtract)
            sd = wp.tile([128, 1], f32, tag="sd")
            nc.scalar.activation(out=sd[:], in_=var[:], func=AF.Sqrt,
                                 bias=eps_t[:, 0:1], scale=1.0)
            rstd = wp.tile([128, 1], f32, tag="rstd")
            nc.vector.reciprocal(out=rstd[:], in_=sd[:])
            scal = wp.tile([128, 1], f32, tag="scal")
            nc.vector.tensor_tensor(out=scal[:], in0=rstd[:],
                                    in1=gam_t[:, l:l + 1], op=OP.mult)
            shif = wp.tile([128, 1], f32, tag="shif")
            nc.vector.tensor_tensor(out=shif[:], in0=mu[:], in1=scal[:],
                                    op=OP.mult)
            nc.vector.tensor_tensor(out=shif[:], in0=bet_t[:, l:l + 1],
                                    in1=shif[:], op=OP.subtract)

            # ----- BN apply + relu + residual + transpose back
            for gi in range(NGRP):
                gw = min(GRP, WPC - gi * GRP)
                wid = gw * 128
                sl = slice(gi * GRP * 128, gi * GRP * 128 + wid)
                act = wp.tile([128, GRP * 128], bf16, tag="act")
                nc.scalar.activation(out=act[:, :wid], in_=hnT[:, sl],
                                     func=AF.Relu, scale=scal[:, 0:1],
                                     bias=shif[:, 0:1])
                nc.vector.tensor_tensor(out=xT[:, sl], in0=act[:, :wid],
                                        in1=xT[:, sl], op=OP.add)
                for wi in range(gw):
                    w = gi * GRP + wi
                    tpp = pt.tile([128, 128], bf16, tag="tp16")
                    nc.tensor.transpose(out=tpp[:],
                                        in_=xT[:, w * 128:(w + 1) * 128],
                                        identity=ident_t[:])
                    nmt = wp.tile([128, 128], bf16, tag="nmb")
                    if l < L - 1:
                        nc.scalar.activation(out=nmt[:], in_=tpp[:],
                                             func=AF.Copy,
                                             scale=invdo_t[:, w:w + 1])
                        xr_write(l + 1, w, nmt)
                    else:
                        nc.scalar.activation(out=nmt[:], in_=tpp[:],
                                             func=AF.Copy)
                        gm = wp.tile([128, G], bf16, tag="gm")
                        nc.vector.tensor_scalar(out=gm[:], in0=iota_t[:],
                                                scalar1=gid_t[:, w:w + 1],
                                                scalar2=None, op0=OP.is_equal)
                        nc.tensor.matmul(out=hgp[:], lhsT=nmt[:], rhs=gm[:],
                                         start=(w == 0), stop=(w == WPC - 1))


        # ------------------------------------------------------------ readout
        hgs = wp.tile([128, G], f32, tag="hgs")
        nc.vector.tensor_copy(out=hgs[:], in_=hgp[:])
        nc.sync.dma_start(out=hg_in[:, :], in_=hgs[:])
        nc.gpsimd.collective_compute(
            "AllReduce", mybir.AluOpType.add, replica_groups=RG,
            ins=[hg_in.ap().opt()], outs=[hg_out.ap().opt()])
        hga = wp.tile([128, G], f32, tag="hga")
        nc.sync.dma_start(out=hga[:], in_=hg_out[:, :])
        nc.vector.tensor_tensor(out=hga[:], in0=hga[:], in1=recip_t[:],
                                op=OP.mult)
        t1p = pp2.tile([64, G], f32, tag="hnp")
        nc.tensor.matmul(out=t1p[:], lhsT=w1_t[:], rhs=hga[:],
                         start=True, stop=True)
        t1 = wp.tile([64, G], f32, tag="t1")
        nc.scalar.activation(out=t1[:], in_=t1p[:], func=AF.Relu,
                             bias=b1_t[:, 0:1], scale=1.0)
        t2p = pp2.tile([32, G], f32, tag="hnp")
        nc.tensor.matmul(out=t2p[:], lhsT=w2_t[:], rhs=t1[:],
                         start=True, stop=True)
        t2 = wp.tile([32, G], f32, tag="t2")
        nc.scalar.activation(out=t2[:], in_=t2p[:], func=AF.Relu,
                             bias=b2_t[:, 0:1], scale=1.0)
        t3p = pp2.tile([16, G], f32, tag="hnp")
        nc.tensor.matmul(out=t3p[:10, :], lhsT=w3_t[:], rhs=t2[:],
                         start=True, stop=True)
        ot = wp.tile([16, G], f32, tag="ot")
        nc.vector.tensor_scalar_add(out=ot[:10, :], in0=t3p[:10, :],
                                    scalar1=b3_t[:, 0:1])
        nc.sync.dma_start(out=out_d[:, :], in_=ot[:10, :])

        for p in [pt, pp3, pp2, pp, gpool, mp, wp, cp]:
            p.release()

    nc.compile()
    return nc


# ------------------------------------------------------------------- kernel()
def kernel(**inputs):
    _apply_patches()
    import jax
    jax.devices()
    _install_ntff_noop()
    import ml_dtypes
    from concourse.bass_utils import run_bass_kernel_spmd

    bf = ml_dtypes.bfloat16
    h = np.asarray(inputs["h"], np.float32)
    src = np.asarray(inputs["src"])
    dst = np.asarray(inputs["dst"])
    graph_id = np.asarray(inputs["graph_id"])

    plan = build_plan(src, dst, graph_id)
    nc = build_program(plan)

    iota = np.tile(np.arange(128, dtype=np.float32)[None, :], (128, 1))
    iotaA = np.tile(iota, (1, plan["maxspan"]))
    iotaB = iotaA + 128.0
    ident = np.eye(128, dtype=np.float32)

    Wl = np.asarray(inputs["Wl"], np.float32)
    in_maps = []
    for r in range(NC):
        hp = np.zeros((VP, HID), np.float32)
        hp[:V] = h[r * V:(r + 1) * V]
        m = {
            "hT_shard": np.ascontiguousarray(hp.T).astype(bf),
            "idx16": plan["idx16"][r],
            "dcol": plan["dcol"][r].astype(bf),
            "invdi": np.tile(plan["invdi"][r][None, :], (128, 1)).astype(bf),
            "invdo": plan["invdo"][r],
            "gid": plan["gid"][r],
            "recip": plan["recip"],
            "iota": iota.astype(bf),
            "iotaA": iotaA.astype(bf),
            "iotaB": iotaB.astype(bf),
            "ident": ident.astype(bf),
            "W_embed": np.asarray(inputs["W_embed"], np.float32).astype(bf),
            "b_embed": np.asarray(inputs["b_embed"], np.float32).reshape(HID, 1),
            "gammas": np.asarray(inputs["gamma"], np.float32).T.copy(),
            "betas": np.asarray(inputs["beta"], np.float32).T.copy(),
            "W1": np.asarray(inputs["W1"], np.float32),
            "b1": np.asarray(inputs["b1"], np.float32).reshape(-1, 1),
            "W2": np.asarray(inputs["W2"], np.float32),
            "b2": np.asarray(inputs["b2"], np.float32).reshape(-1, 1),
            "W3": np.asarray(inputs["W3"], np.float32),
            "b3": np.asarray(inputs["b3"], np.float32).reshape(-1, 1),
        }
        for i in range(L):
            m[f"Wl{i}"] = Wl[i].astype(bf)
        in_maps.append(m)

    trace = os.environ.get("GCN_TRACE") == "1"
    res = run_bass_kernel_spmd(nc, in_maps, core_ids=list(range(NC)),
                               trace=trace)
    if trace and res.exec_time_ns:
        print(f"HW exec time: {res.exec_time_ns} ns")
        if res.instructions_and_trace:
            print("trace:", res.instructions_and_trace[1])
    return np.ascontiguousarray(res.results[0]["out"].T)


def _install_ntff_noop():
    """bass_utils imports antenv.axon_hooks unconditionally when trace=True;
    provide the module (and, for GCN_TRACE=1, the real ctypes hook)."""
    if "antenv.axon_hooks" in sys.modules:
        return
    mod = types.ModuleType("antenv.axon_hooks")
    _h = [None]
    mod.set_axon_ntff_profile_hook = lambda h: _h.__setitem__(0, h)
    mod.get_axon_ntff_profile_hook = lambda: _h[0]
    sys.modules["antenv.axon_hooks"] = mod
    try:
        import antenv
        antenv.axon_hooks = mod
    except ImportError:
        pass
    if os.environ.get("GCN_TRACE") == "1":
        try:
            import ctypes
            from contextlib import contextmanager

            lib = ctypes.CDLL("/opt/axon/libaxon_pjrt.so")
            lib.axon_start_nrt_profile.argtypes = [
                ctypes.POINTER(ctypes.c_int64), ctypes.c_size_t]
            lib.axon_start_nrt_profile.restype = ctypes.c_int64
            lib.axon_stop_nrt_profile.argtypes = [ctypes.c_char_p]
            lib.axon_stop_nrt_profile.restype = ctypes.c_int64

            @contextmanager
            def ntff_profile(output_dir, device_ids=None):
                if device_ids:
                    ids = (ctypes.c_int64 * len(device_ids))(*device_ids)
                    rc = lib.axon_start_nrt_profile(ids, len(device_ids))
                else:
                    rc = lib.axon_start_nrt_profile(None, 0)
                if rc < 0:
                    raise RuntimeError(f"axon_start_nrt_profile rc={rc}")
                try:
                    yield
                finally:
                    n = lib.axon_stop_nrt_profile(str(output_dir).encode())
                    if n < 0:
                        raise RuntimeError(f"axon_stop_nrt_profile rc={n}")

            mod.set_axon_ntff_profile_hook(ntff_profile)
        except Exception:
            pass
